# revision 8
# baseline (speedup 1.0000x reference)
"""Performer/FAVOR+ attention for Trainium2 — single fused NEFF, SPMD x8.

Wire-optimized: the axon tunnel moves ~45 MB/s, so the kernel minimizes
host<->device bytes.  Inputs ship as native fp16 NEFF tensors (x rows +
1/8 weight-stack shard packed in one array, compact cos/sin in another);
fp16 matmuls accumulate in fp32 on the PE, so this costs only the fp16
rounding of the wire itself.  The single NEFF does: AllGather of the
fp16 weight shards, QKV projections + rotary, FAVOR features, per-head
kvs partials, a 4-core-group AllReduce (cores 0-3 = batch 0, 4-7 =
batch 1), and the output projection.  Output returns as row-scaled
uint8 (err <= rowmax/254) plus a per-row fp32 scale — 8 MB instead of
32 MB fp32.

EPS handling: adding EPS to the exp features *before* the kvs matmul
reproduces exactly the reference's (exp+EPS) keys — kvs += EPS*vsum and
ks += EPS*L fall out of the contraction against [v | 1] — so no host
fixup of the reduced kvs is needed.  The query-side EPS is the rank-1
epst correction in the num/den matmul.
"""

import sys

sys.path.insert(0, "/opt/trn_rl_repo")

from contextlib import ExitStack

import numpy as np

import jax
import jax.numpy as jnp
from jax.sharding import Mesh, PartitionSpec, NamedSharding
from jax.experimental.shard_map import shard_map

import concourse.bass as bass
import concourse.mybir as mybir
import concourse.tile as tile
from concourse import bacc
from concourse.bass import ts
from concourse.bass2jax import (
    _bass_exec_p,
    install_neuronx_cc_hook,
    partition_id_tensor,
)
from concourse.masks import make_identity

BF = mybir.dt.bfloat16
F16 = mybir.dt.float16
F32 = mybir.dt.float32
FR = mybir.dt.float32r
U8 = mybir.dt.uint8
ACT_COPY = mybir.ActivationFunctionType.Copy
ACT_EXP = mybir.ActivationFunctionType.Exp
ACT_ABS = mybir.ActivationFunctionType.Abs

B, L, DM = 2, 4096, 1024
H, DH, M = 16, 64, 256
ROWS = 1024
RT = ROWS // 128
KT = DM // 128
C1 = float(DH) ** -0.25
EPS = 1e-6
WSL = (4 * DM + 128) // 8   # 528 stacked-weight rows per core
PKR = ROWS + WSL            # packed input rows per core

_CACHE = {}


def _tr4(nc, tr_pool, dst_ap, srcs, ident, dtype):
    """Transpose four [128,128] blocks through one psum tile, one evict."""
    ps = tr_pool.tile([128, 512], dtype, name="trps", tag="tr")
    for i, s in enumerate(srcs):
        nc.tensor.transpose(ps[:, ts(i, 128)], s, ident)
    nc.scalar.activation(dst_ap, ps[:], ACT_COPY)


def _load_xt(nc, tc, ctx, pk, xt, ident16, tr_pool):
    xin = ctx.enter_context(tc.tile_pool(name="xin", bufs=2))
    for rt in range(RT):
        xrow = xin.tile([128, DM], F16, name="xrow")
        nc.sync.dma_start(xrow[:], pk[ts(rt, 128), :])
        for g in range(2):
            _tr4(nc, tr_pool, xt[:, g * 4:(g + 1) * 4, ts(rt, 128)],
                 [xrow[:, ts(g * 4 + i, 128)] for i in range(4)], ident16, F16)


def _cossin(nc, cs16, cspool, rt):
    """Expand compact [128,128] f16 (cos64|sin64) to f32 [128, DM] tables."""
    csb = cspool.tile([128, 128], F16, name="csb", tag="csb")
    nc.sync.dma_start(csb[:], cs16[ts(rt, 128), :])
    cost = cspool.tile([128, DM], FR, name="cost", tag="cos")
    sint = cspool.tile([128, DM], FR, name="sint", tag="sin")
    for h in range(H):
        nc.scalar.activation(cost[:, ts(h, 64)], csb[:, 0:64], ACT_COPY)
        nc.scalar.activation(sint[:, ts(h, 64)], csb[:, 64:128], ACT_COPY)
    return cost, sint


def _proj_rotary(nc, xt, wall, wbase, cost, sint, mm_pool, rot_pool, wpool,
                 rt, do_rotary):
    ps = [mm_pool.tile([128, 512], F32, name=f"mmps{i}", tag="mm")
          for i in range(2)]
    for nt in range(2):
        for k in range(KT):
            wtile = wpool.tile([128, 512], F16, name="wtile")
            nc.sync.dma_start(
                wtile[:],
                wall[wbase + k * 128:wbase + (k + 1) * 128, ts(nt, 512)])
            nc.tensor.matmul(ps[nt][:], xt[:, k, ts(rt, 128)], wtile[:],
                             start=(k == 0), stop=(k == KT - 1))
    raw = rot_pool.tile([128, DM], FR, name="raw", tag="raw")
    for nt in range(2):
        nc.scalar.activation(raw[:, ts(nt, 512)], ps[nt][:], ACT_COPY)
    if not do_rotary:
        return raw
    # interleaved rotate_every_two: tmp[2i] = raw[2i+1], tmp[2i+1] = raw[2i]
    # (the sign of the sin table handles the negation)
    r3 = raw.rearrange("p (hd two) -> p hd two", two=2)
    tmp = rot_pool.tile([128, H, 64], FR, name="tmp", tag="tmp")
    t3 = tmp.rearrange("p h (hd two) -> p (h hd) two", two=2)
    nc.vector.tensor_copy(t3[:, :, 0:1], r3[:, :, 1:2])
    nc.vector.tensor_copy(t3[:, :, 1:2], r3[:, :, 0:1])
    m1 = rot_pool.tile([128, DM], FR, name="m1", tag="m1")
    nc.vector.tensor_mul(m1[:], raw[:], cost[:])
    m2 = rot_pool.tile([128, DM], FR, name="m2", tag="m2")
    nc.vector.tensor_mul(m2[:], tmp.rearrange("p h d -> p (h d)"), sint[:])
    nc.vector.tensor_add(m1[:], m1[:], m2[:])
    return m1


def _diag16(nc, small_pool, rot_pool, rot):
    sq = rot_pool.tile([128, DM], F32, name="sq", tag="m2")
    nc.vector.tensor_mul(sq[:], rot[:], rot[:])
    d16 = small_pool.tile([128, H], F32, name="d16", tag="d16")
    nc.vector.tensor_reduce(d16[:], sq.rearrange("p (h d) -> p h d", d=64),
                            axis=mybir.AxisListType.X, op=mybir.AluOpType.add)
    return d16


def _dd_rowtile(nc, krt_pool, tr_pool, dd_pool, ddsb_pool, rot, projt2,
                identf):
    """dd_sb [128, H*M] fp32 = per-head rot @ projT (raw, no C1)."""
    krt = krt_pool.tile([128, KT, 128], FR, name="krt", tag="krt")
    for g in range(2):
        _tr4(nc, tr_pool, krt[:, g * 4:(g + 1) * 4, :],
             [rot[:, ts(g * 4 + i, 128)] for i in range(4)], identf, FR)
    dd_sb = ddsb_pool.tile([128, H * M], F32, name="dd_sb", tag="ddsb")
    for quarter in range(4):
        ddps = dd_pool.tile([128, 1024], F32, name="ddps", tag="dd")
        for i in range(2):
            hp = quarter * 2 + i
            nc.tensor.matmul(ddps[:, ts(i, 512)], krt[:, hp, :],
                             projt2[:, :], start=True, stop=True)
        nc.scalar.activation(dd_sb[:, ts(quarter, 1024)], ddps[:], ACT_COPY)
    return dd_sb


def build_fused():
    nc = bacc.Bacc(None, target_bir_lowering=False, num_devices=8)
    pk = nc.dram_tensor("pk", [PKR, DM], F16, kind="ExternalInput")
    cs16 = nc.dram_tensor("cs16", [ROWS, 128], F16, kind="ExternalInput")
    outc = nc.dram_tensor("outc", [ROWS, DM], U8, kind="ExternalOutput")
    sclc = nc.dram_tensor("sclc", [ROWS, 1], F32, kind="ExternalOutput")

    with tile.TileContext(nc) as tc, ExitStack() as ctx:
        const = ctx.enter_context(tc.tile_pool(name="const", bufs=1))
        tr_pool = ctx.enter_context(
            tc.tile_pool(name="trps", bufs=2, space="PSUM"))
        mm_pool = ctx.enter_context(
            tc.tile_pool(name="mmps", bufs=4, space="PSUM"))
        dd_pool = ctx.enter_context(
            tc.tile_pool(name="ddps", bufs=1, space="PSUM"))
        wpool = ctx.enter_context(tc.tile_pool(name="w", bufs=6))
        cspool = ctx.enter_context(tc.tile_pool(name="cs", bufs=2))
        rot_pool = ctx.enter_context(tc.tile_pool(name="rot", bufs=2))
        small = ctx.enter_context(tc.tile_pool(name="small", bufs=3))
        krt_pool = ctx.enter_context(tc.tile_pool(name="krt", bufs=2))
        ddsb_pool = ctx.enter_context(tc.tile_pool(name="ddsb", bufs=1))
        dram = ctx.enter_context(tc.tile_pool(name="dram", bufs=1,
                                              space="DRAM"))

        ident32 = const.tile([128, 128], F32)
        make_identity(nc, ident32[:])
        identf = const.tile([128, 128], FR)
        nc.scalar.activation(identf[:], ident32[:], ACT_COPY)
        ident16 = const.tile([128, 128], F16)
        make_identity(nc, ident16[:])

        # --- AllGather the fp16 weight stack: 528 rows/core -> 4224 ---
        # collective outputs live in Shared scratchpad (the supported
        # fast path for HBM-HBM collectives); inputs must stay Local
        wslb = dram.tile([WSL, DM], F16, name="wslb")
        wall = nc.dram_tensor("wall", [8 * WSL, DM], F16,
                              addr_space="Shared")
        nc.gpsimd.dma_start(wslb[:], pk[ROWS:PKR, :])
        nc.gpsimd.collective_compute(
            "AllGather", mybir.AluOpType.bypass,
            replica_groups=[[0, 1, 2, 3, 4, 5, 6, 7]],
            ins=[wslb[:].opt()], outs=[wall[:].opt()])
        WQ0, WK0, WV0, WO0, PJ0 = 0, DM, 2 * DM, 3 * DM, 4 * DM

        pj16 = const.tile([128, 2 * M], F16)
        nc.sync.dma_start(pj16[:], wall[PJ0:PJ0 + 128, 0:2 * M])
        projt2 = const.tile([128, 2 * M], FR)
        nc.scalar.activation(projt2[:], pj16[:], ACT_COPY)

        kva = const.tile([128, H, 2, 65], F16)
        epst = const.tile([1, H, 65], F16)
        epsones = const.tile([1, 128], F16)
        nc.any.memset(epsones[:], 1.0)
        ones_col = const.tile([128, 1], F16)
        nc.any.memset(ones_col[:], 1.0)

        xt_pool = ctx.enter_context(tc.tile_pool(name="xt", bufs=1))
        xt = xt_pool.tile([128, KT, ROWS], F16)
        _load_xt(nc, tc, ctx, pk, xt, ident16, tr_pool)

        kvb = dram.tile([H * 2 * 128, 65], F32, name="kvb")
        kvr = dram.tile([H * 2 * 128, 65], F32, name="kvr")

        # ---------------- phase 1: keys/values of own rows ----------------
        with ExitStack() as p1:
            ek_pool = p1.enter_context(tc.tile_pool(name="ek", bufs=1))
            va_pool = p1.enter_context(tc.tile_pool(name="va", bufs=1))
            kv_pool = p1.enter_context(tc.tile_pool(name="kv", bufs=2))
            ek = ek_pool.tile([128, RT, H, M], F16)
            vaug = va_pool.tile([128, RT, H, 65], F16)

            for rt in range(RT):
                cost, sint = _cossin(nc, cs16, cspool, rt)

                kr = _proj_rotary(nc, xt, wall, WK0, cost, sint, mm_pool,
                                  rot_pool, wpool, rt, True)
                d16 = _diag16(nc, small, rot_pool, kr)

                v = _proj_rotary(nc, xt, wall, WV0, cost, sint, mm_pool,
                                 rot_pool, wpool, rt, False)
                nc.vector.tensor_copy(vaug[:, rt, :, 0:64],
                                      v.rearrange("p (h d) -> p h d", d=64))
                nc.any.memset(vaug[:, rt, :, 64:65], 1.0)

                dd_sb = _dd_rowtile(nc, krt_pool, tr_pool, dd_pool,
                                    ddsb_pool, kr, projt2, identf)

                mk = small.tile([128, 1], F32, name="mk", tag="mk")
                nc.vector.tensor_reduce(mk[:], dd_sb[:],
                                        axis=mybir.AxisListType.X,
                                        op=mybir.AluOpType.max)
                mks = small.tile([128, 1], F32, name="mks", tag="mks")
                nc.vector.tensor_scalar_mul(mks[:], mk[:], C1)
                negb = small.tile([128, H], F32, name="negb", tag="negb")
                nc.vector.tensor_scalar(negb[:], d16[:], -0.5 * C1 * C1,
                                        mks[:], op0=mybir.AluOpType.mult,
                                        op1=mybir.AluOpType.subtract)
                for h in range(H):
                    nc.scalar.activation(ek[:, rt, h, :],
                                         dd_sb[:, ts(h, 256)], ACT_EXP,
                                         bias=negb[:, h:h + 1], scale=C1)
                # key features are exp(...) + EPS; folding EPS here makes
                # the kvs contraction emit the EPS*vsum / EPS*L terms.
                nc.vector.tensor_scalar_add(ek[:, rt, :, :], ek[:, rt, :, :],
                                            EPS)

            # kvs^T partials: [m, (v | 1)] per head, accumulated over rows
            for h in range(H):
                kps = mm_pool.tile([128, 512], F32, name="kps", tag="mm")
                for j in range(2):
                    for rt in range(RT):
                        nc.tensor.matmul(kps[:, j * 65:(j + 1) * 65],
                                         ek[:, rt, h, ts(j, 128)],
                                         vaug[:, rt, h, :],
                                         start=(rt == 0), stop=(rt == RT - 1))
                ksb = kv_pool.tile([128, 130], F32, name="ksb")
                nc.scalar.activation(ksb[:], kps[:, 0:130], ACT_COPY)
                for j in range(2):
                    nc.sync.dma_start(
                        kvb[(h * 2 + j) * 128:(h * 2 + j + 1) * 128, :],
                        ksb[:, j * 65:(j + 1) * 65])

        # ------- reduce kvs over the 4 cores that share each batch -------
        nc.gpsimd.collective_compute(
            "AllReduce", mybir.AluOpType.add,
            replica_groups=[[0, 1, 2, 3], [4, 5, 6, 7]],
            ins=[kvb[:].opt()], outs=[kvr[:].opt()])

        kv2_pool = ctx.enter_context(tc.tile_pool(name="kv2", bufs=2))
        for h in range(H):
            eps_ps = mm_pool.tile([1, 65], F32, name="epsps", tag="mm")
            for j in range(2):
                ktmp = kv2_pool.tile([128, 65], F32, name="ktmp")
                nc.sync.dma_start(
                    ktmp[:],
                    kvr[(h * 2 + j) * 128:(h * 2 + j + 1) * 128, :])
                nc.scalar.activation(kva[:, h, j, :], ktmp[:], ACT_COPY)
                nc.tensor.matmul(eps_ps[:], ones_col[:], kva[:, h, j, :],
                                 start=(j == 0), stop=(j == 1))
            nc.scalar.activation(epst[:, h, :], eps_ps[:], ACT_COPY,
                                 scale=EPS)

        # ---------------- phase 2: queries of own rows -------------------
        qp_pool = ctx.enter_context(tc.tile_pool(name="qp", bufs=2))
        qpt_pool = ctx.enter_context(tc.tile_pool(name="qpt", bufs=2))
        ns_pool = ctx.enter_context(tc.tile_pool(name="ns", bufs=2))
        av_pool = ctx.enter_context(tc.tile_pool(name="av", bufs=2))
        avt_pool = ctx.enter_context(tc.tile_pool(name="avt", bufs=2))
        out_pool = ctx.enter_context(tc.tile_pool(name="osb", bufs=2))

        for rt in range(RT):
            cost, sint = _cossin(nc, cs16, cspool, rt)

            qr = _proj_rotary(nc, xt, wall, WQ0, cost, sint, mm_pool,
                              rot_pool, wpool, rt, True)
            d16 = _diag16(nc, small, rot_pool, qr)
            dd_sb = _dd_rowtile(nc, krt_pool, tr_pool, dd_pool, ddsb_pool,
                                qr, projt2, identf)

            mq = small.tile([128, H], F32, name="mq", tag="mq")
            nc.vector.tensor_reduce(mq[:],
                                    dd_sb.rearrange("p (h m) -> p h m", m=M),
                                    axis=mybir.AxisListType.X,
                                    op=mybir.AluOpType.max)
            mqs = small.tile([128, H], F32, name="mqs", tag="mqs")
            nc.vector.tensor_scalar_mul(mqs[:], mq[:], C1)
            negb = small.tile([128, H], F32, name="negb", tag="negb")
            nc.vector.tensor_scalar(negb[:], d16[:], -0.5 * C1 * C1, None,
                                    op0=mybir.AluOpType.mult)
            nc.vector.tensor_tensor(negb[:], negb[:], mqs[:],
                                    op=mybir.AluOpType.subtract)
            qp = qp_pool.tile([128, H * M], F16, name="qp", tag="qp")
            for h in range(H):
                nc.scalar.activation(qp[:, ts(h, 256)], dd_sb[:, ts(h, 256)],
                                     ACT_EXP, bias=negb[:, h:h + 1], scale=C1)

            qpt = qpt_pool.tile([128, H, 2, 128], F16, name="qpt", tag="qpt")
            for g in range(8):
                _tr4(nc, tr_pool,
                     qpt[:, g * 2:(g + 1) * 2, :, :],
                     [qp[:, ts(g * 4 + i, 128)] for i in range(4)],
                     ident16, F16)

            nsb = ns_pool.tile([128, H, 65], F32, name="nsb", tag="ns")
            for quarter in range(4):
                nps = mm_pool.tile([128, 260], F32, name="nps", tag="mm")
                for i in range(4):
                    h = quarter * 4 + i
                    for j in range(2):
                        nc.tensor.matmul(nps[:, ts(i, 65)],
                                         qpt[:, h, j, :], kva[:, h, j, :],
                                         start=(j == 0), stop=False)
                    # += EPS*colsum(kva): rank-1 via K=1 matmul
                    nc.tensor.matmul(nps[:, ts(i, 65)], epsones[:, :],
                                     epst[:, h, :], start=False, stop=True)
                nc.scalar.activation(
                    nsb[:, quarter * 4:(quarter + 1) * 4, :].rearrange(
                        "p h f -> p (h f)"), nps[:], ACT_COPY)

            den = small.tile([128, H], F32, name="den", tag="den")
            nc.vector.tensor_copy(den[:], nsb[:, :, 64])
            rden = small.tile([128, H], F32, name="rden", tag="rden")
            nc.vector.reciprocal(rden[:], den[:])
            av = av_pool.tile([128, H, 64], F16, name="av", tag="av")
            for h in range(H):
                nc.vector.tensor_scalar_mul(av[:, h, :], nsb[:, h, 0:64],
                                            rden[:, h:h + 1])

            avt = avt_pool.tile([128, KT, 128], F16, name="avt", tag="avt")
            av2 = av.rearrange("p h d -> p (h d)")
            for g in range(2):
                _tr4(nc, tr_pool, avt[:, g * 4:(g + 1) * 4, :],
                     [av2[:, ts(g * 4 + i, 128)] for i in range(4)],
                     ident16, F16)
            osb32 = out_pool.tile([128, DM], F32, name="osb32", tag="osb32")
            for nt in range(2):
                ops = mm_pool.tile([128, 512], F32, name="ops", tag="mm")
                for k in range(KT):
                    wtile = wpool.tile([128, 512], F16, name="wotile",
                                       tag="wo")
                    nc.sync.dma_start(
                        wtile[:],
                        wall[WO0 + k * 128:WO0 + (k + 1) * 128, ts(nt, 512)])
                    nc.tensor.matmul(ops[:], avt[:, k, :], wtile[:],
                                     start=(k == 0), stop=(k == KT - 1))
                nc.scalar.activation(osb32[:, ts(nt, 512)], ops[:], ACT_COPY)

            # row-scaled uint8 quantization: u8 = round(x*127/amax) + 128
            oab = out_pool.tile([128, DM], F32, name="oab", tag="oab")
            nc.scalar.activation(oab[:], osb32[:], ACT_ABS)
            amax = small.tile([128, 1], F32, name="amax", tag="amax")
            nc.vector.tensor_reduce(amax[:], oab[:],
                                    axis=mybir.AxisListType.X,
                                    op=mybir.AluOpType.max)
            rcp = small.tile([128, 1], F32, name="rcp", tag="rcp")
            nc.vector.reciprocal(rcp[:], amax[:])
            r127 = small.tile([128, 1], F32, name="r127", tag="r127")
            nc.vector.tensor_scalar_mul(r127[:], rcp[:], 127.0)
            u8 = out_pool.tile([128, DM], U8, name="u8", tag="u8")
            nc.vector.tensor_scalar(u8[:], osb32[:], r127[:], 128.0,
                                    op0=mybir.AluOpType.mult,
                                    op1=mybir.AluOpType.add)
            nc.sync.dma_start(outc[ts(rt, 128), :], u8[:])
            scl = small.tile([128, 1], F32, name="scl", tag="scl")
            nc.vector.tensor_scalar_mul(scl[:], amax[:], 1.0 / 127.0)
            nc.sync.dma_start(sclc[ts(rt, 128), :], scl[:])

    nc.compile()
    return nc


def _prep_pk(x, Wq, Wk, Wv, Wo, proj):
    """fp16 wire: pk [8*PKR, DM] = per-core (x rows | weight shard)."""
    if "pk" not in _CACHE:
        _CACHE["pk"] = np.empty((8 * PKR, DM), np.float16)
        _CACHE["wg16"] = np.empty((8 * WSL, DM), np.float16)
        _CACHE["cs"] = np.empty((2 * L, 128), np.float16)
    pk = _CACHE["pk"]
    wg = _CACHE["wg16"]
    pk3 = pk.reshape(8, PKR, DM)
    pk3[:, 0:ROWS] = x.reshape(8, ROWS, DM)

    wg[0:DM] = Wq
    wg[DM:2 * DM] = Wk
    wg[2 * DM:3 * DM] = Wv
    wg[3 * DM:4 * DM] = Wo
    pj = wg[4 * DM:4 * DM + 128]
    pj[:] = 0
    pj[0:64, 0:M] = proj.T
    pj[64:128, M:2 * M] = proj.T
    pk3[:, ROWS:PKR] = wg.reshape(8, WSL, DM)
    return pk


def _prep_cs(sinu_pos):
    """interleaved rotary tables: cos/sin repeated 2x, sin sign-baked."""
    cs = _CACHE["cs"]
    sp = sinu_pos.reshape(L, 2, 32)
    cs[0:L, 0:64:2] = sp[:, 1, :]
    cs[0:L, 1:64:2] = sp[:, 1, :]
    cs[0:L, 64:128:2] = -sp[:, 0, :]
    cs[0:L, 65:128:2] = sp[:, 0, :]
    cs[L:2 * L] = cs[0:L]
    return cs


def _get_runner():
    if "runner" in _CACHE:
        return _CACHE["runner"]
    install_neuronx_cc_hook()
    nc = build_fused()

    partition_name = (nc.partition_id_tensor.name
                      if nc.partition_id_tensor else None)
    in_names, out_names, out_avals = [], [], []
    for alloc in nc.m.functions[0].allocations:
        if not isinstance(alloc, mybir.MemoryLocationSet):
            continue
        name = alloc.memorylocations[0].name
        if alloc.kind == "ExternalInput":
            if name != partition_name:
                in_names.append(name)
        elif alloc.kind == "ExternalOutput":
            out_names.append(name)
            out_avals.append(jax.core.ShapedArray(
                tuple(alloc.tensor_shape), mybir.dt.np(alloc.dtype)))
    assert in_names == ["pk", "cs16"], in_names
    assert out_names == ["outc", "sclc"], out_names
    all_in = in_names + out_names + ([partition_name] if partition_name
                                     else [])

    devices = jax.devices()[:8]
    mesh = Mesh(np.asarray(devices), ("core",))
    P = PartitionSpec
    sh = NamedSharding(mesh, P("core"))

    def run(pk, cs, z1, z2):
        operands = [pk, cs, z1, z2]
        if partition_name:
            operands.append(partition_id_tensor())
        outs = _bass_exec_p.bind(
            *operands, out_avals=tuple(out_avals), in_names=tuple(all_in),
            out_names=tuple(out_names), lowering_input_output_aliases=(),
            sim_require_finite=True, sim_require_nnan=True, nc=nc)
        return tuple(outs)

    jit2 = jax.jit(shard_map(
        run, mesh=mesh, in_specs=(P("core"),) * 4,
        out_specs=(P("core"),) * 2, check_rep=False),
        donate_argnums=(0, 1), keep_unused=True)

    # persistent zero stand-ins for the output operands (never donated,
    # never read: the NEFF writes every element of both outputs)
    z1 = jax.device_put(np.zeros((8 * ROWS, DM), np.uint8), sh)
    z2 = jax.device_put(np.zeros((8 * ROWS, 1), np.float32), sh)
    z1.block_until_ready()
    z2.block_until_ready()

    _CACHE["runner"] = (jit2, z1, z2, sh)
    return _CACHE["runner"]


def kernel(x, Wq, Wk, Wv, Wo, proj, sinu_pos):
    f32 = np.float32
    x, Wq, Wk, Wv, Wo, proj = (np.asarray(a, f32)
                               for a in (x, Wq, Wk, Wv, Wo, proj))
    sinu = np.asarray(sinu_pos, f32).reshape(L, DH)
    for attempt in range(3):
        try:
            jit2, z1, z2, sh = _get_runner()
            pk = _prep_pk(x, Wq, Wk, Wv, Wo, proj)
            pk_dev = jax.device_put(pk, sh)   # async: wire starts now
            cs = _prep_cs(sinu)               # overlaps the pk transfer
            u8, scl = jit2(pk_dev, cs, z1, z2)
            u8.copy_to_host_async()
            scl.copy_to_host_async()
            scl = np.asarray(scl)
            u8 = np.asarray(u8)
            break
        except Exception:
            if attempt == 2:
                raise
            # tunnel worker may have died — reset the backend and re-jit
            _CACHE.pop("runner", None)
            try:
                jax.clear_backends()
            except Exception:
                pass
    out = np.subtract(u8, np.float32(128.0), dtype=f32)
    out *= scl
    try:
        jax.effects_barrier()
    except Exception:
        pass
    return np.ascontiguousarray(out.reshape(B, L, DM))
